# revision 32
# baseline (speedup 1.0000x reference)
"""Trainium2 Bass kernel for the 4-layer spiking-MLP critic (T=16 IF/LIF recurrence).

Strategy (v2: all-fp8 DoubleRow weight cascades)
- Data-parallel over 8 NeuronCores: batch 4096 -> 512 per core; weights replicated,
  tensors pre-arranged on the host into [partition, k, free] layouts.
- Everything runs transposed (feature dim on partitions, batch on the free dim).
- dv1 = x @ W1.T + b1 is time-invariant: computed once at startup with the
  f16 hi/lo 3-chain scheme (xh@w1h + xls@w1h + xhs@w1l ~= 2^-22), overlapping
  the steady-state weight DMA stream.
- W2/W3 are stored as residual cascades of e4m3 tensors, each contracted with a
  DoubleRow fp8 matmul chain (0.5 cyc/row, 2 k-tiles per instruction = 1/4 the
  cost of an f16 chain).  Stage j stores e4m3((w - prev) * 2^s_j) and is paired
  with a moving spike copy s * 2^-s_j held in e5m2 (spikes are 0/1 so every
  scaled copy is exact; all scales are e5m2-normal).
    W2: scales {0,4,8,12,16} -> ~2^-21 worst-case weight error (5 chains)
    W3: scales {0,4,8,12}    -> ~2^-17 (4 chains)
  This beats the f16-hi+lo decomposition frontier: same precision as the
  previous f16-based kernel at 9.25 vs 11.25 chain-units per step.
- W4 (feeds the non-spiking output LIF): e4m3 + e5m2 residual DoubleRow pair
  reading the e4m3 spike tile directly (error here is negligible, measured).
- IF update per layer-step: update (DVE stt psum+bias+v), spike (Pool is_ge
  written straight to fp8), scaled copies (Act Identity-with-scale), reset
  (DVE stt min/subtract).  PSUM drains in [P,2,N] two-bank blocks so reset ops
  batch two c-tiles per instruction.  All resets are deferred to the step tail
  (they soak the next step's DVE slack instead of gating the spike pipeline),
  and the layer-1 integrate is gated on a tiny `ones` tile produced from the
  last layer-2 block so the greedy per-engine scheduler cannot hoist the L1
  chain ahead of the latency-critical final layer-2 spike.
- Layer-4 Horner recurrence in one persistent PSUM bank: zh <- (zh + z_t)*0.5,
  steps t<4 skipped (below f32 rounding of the result); L4(t-1) is emitted as
  PE filler at the layer-2 -> layer-3 seam of step t, and the final step's L4
  chains interleave into the last layer-3 blocks with a one-block lag.
"""

import sys

sys.path.insert(0, "/opt/trn_rl_repo")

import numpy as np
import ml_dtypes

P = 128
D, H, AOUT = 512, 1024, 64
N = 512           # batch per core
T = 16
KD, KH = D // P, H // P
NCORES = 8

S2 = [0, 4, 8, 12, 16]     # W2 cascade stage scales (bits)
S3 = [0, 4, 8, 12]         # W3 cascade stage scales
T4 = 4                     # skip L4 for t < T4 (below f32 rounding)

_CACHE = {}


def _build():
    from contextlib import ExitStack
    from concourse import bacc, mybir, tile

    f32 = mybir.dt.float32
    f8e4 = mybir.dt.float8e4
    f8e5 = mybir.dt.float8e5
    A = mybir.AluOpType
    IDENT = mybir.ActivationFunctionType.Identity
    DR = mybir.MatmulPerfMode.DoubleRow

    nc = bacc.Bacc("TRN2", target_bir_lowering=False, debug=False)

    din = {}
    f16d = mybir.dt.float16
    for name, shape, dt_ in (
        [("xh", [P, KD * N], f16d), ("xls", [P, KD * N], f16d),
         ("xhs", [P, KD * N], f16d),
         ("w1h", [P, KD * H], f16d), ("w1l", [P, KD * H], f16d)]
        + [(f"w2s{j}", [P, KH * H], f8e4) for j in range(len(S2))]
        + [(f"w3s{j}", [P, KH * H], f8e4) for j in range(len(S3))]
        + [("w4h8", [P, KH * AOUT], f8e4), ("w4l8", [P, KH * AOUT], f8e5),
           ("b1", [P, KH], f32), ("b2", [P, KH], f32), ("b3", [P, KH], f32),
           ("b4f", [AOUT, 1], f32)]
    ):
        din[name] = nc.dram_tensor(name, shape, dt_, kind="ExternalInput")
    dout = nc.dram_tensor("v4T", [AOUT, N], f32, kind="ExternalOutput")

    ts = lambda i, sz: slice(i * sz, (i + 1) * sz)

    with tile.TileContext(nc) as tc, ExitStack() as ctx:
        wpool = ctx.enter_context(tc.tile_pool(name="w", bufs=1))
        vpool = ctx.enter_context(tc.tile_pool(name="v", bufs=1))
        spool = ctx.enter_context(tc.tile_pool(name="s", bufs=1))
        mmps = ctx.enter_context(tc.tile_pool(name="mmps", bufs=3, space="PSUM"))
        zps = ctx.enter_context(tc.tile_pool(name="zps", bufs=1, space="PSUM"))

        # ---- state / spike tiles ----
        dv1b = vpool.tile([P, KH, N], f32, tag="dv1b")
        v1 = vpool.tile([P, KH, N], f32, tag="v1")
        v2 = vpool.tile([P, KH, N], f32, tag="v2")
        v3 = vpool.tile([P, KH, N], f32, tag="v3")
        s1c = [spool.tile([P, KH, N], f8e5, name=f"s1c{j}", tag=f"s1c{j}") for j in range(len(S2))]
        s2c = [spool.tile([P, KH, N], f8e5, name=f"s2c{j}", tag=f"s2c{j}") for j in range(len(S3))]
        s3e = spool.tile([P, KH, N], f8e4, tag="s3e")
        zh = zps.tile([AOUT, N], f32, tag="zh")

        # copy engines: j -> engine for the scaled spike copies
        CP_ENG = {1: nc.scalar, 2: nc.scalar, 3: nc.gpsimd, 4: nc.scalar}
        CP_ORDER = [1, 2, 3, 4]

        def l1_spike_half(h, wr, src_v):
            hs = slice(4 * h, 4 * h + 4)
            nc.gpsimd.tensor_scalar(wr[0][:, hs, :], src_v[:, hs, :], 1.0, None, A.is_ge)
            for j in [jj for jj in CP_ORDER if jj < len(S2)]:
                eng = CP_ENG[j]
                if eng is nc.scalar:
                    eng.activation(wr[j][:, hs, :], wr[0][:, hs, :], IDENT,
                                   scale=float(2.0 ** -S2[j]))
                else:
                    eng.tensor_scalar(wr[j][:, hs, :], wr[0][:, hs, :],
                                      float(2.0 ** -S2[j]), None, A.mult)
            nc.vector.scalar_tensor_tensor(v1[:, hs, :], src_v[:, hs, :], 1.0,
                                           wr[0][:, hs, :], A.min, A.subtract)

        # ---- startup: x and W1 (f32, exact), then the steady-state weights ----
        with tc.tile_pool(name="startup", bufs=1) as stp:
            f16 = mybir.dt.float16
            xh = stp.tile([P, KD, N], f16, tag="xh")
            w1h = stp.tile([P, KD, H], f16, tag="w1h")
            xls = stp.tile([P, KD, N], f16, tag="xls")
            xhs = stp.tile([P, KD, N], f16, tag="xhs")
            w1l = stp.tile([P, KD, H], f16, tag="w1l")
            for k in range(KD):
                nc.sync.dma_start(w1h[:, k, :], din["w1h"].ap()[:, ts(k, H)])
                nc.sync.dma_start(xh[:, k, :], din["xh"].ap()[:, ts(k, N)])
            for k in range(KD):
                nc.sync.dma_start(xls[:, k, :], din["xls"].ap()[:, ts(k, N)])
                nc.sync.dma_start(xhs[:, k, :], din["xhs"].ap()[:, ts(k, N)])
                nc.sync.dma_start(w1l[:, k, :], din["w1l"].ap()[:, ts(k, H)])
            b1sb = wpool.tile([P, KH], f32, tag="b1")
            nc.sync.dma_start(b1sb[:], din["b1"].ap())
            b2sb = wpool.tile([P, KH], f32, tag="b2")
            nc.sync.dma_start(b2sb[:], din["b2"].ap())
            b3sb = wpool.tile([P, KH], f32, tag="b3")
            nc.sync.dma_start(b3sb[:], din["b3"].ap())
            b4sb = wpool.tile([AOUT, 1], f32, tag="b4f")
            nc.sync.dma_start(b4sb[:], din["b4f"].ap())
            # steady-state weights stream behind x/w1; k-outer so every stage's
            # first k-pairs land early (the t=0 chains consume kp-major and
            # would otherwise stall on the last stage's DMA)
            w2s = [wpool.tile([P, KH, H], f8e4, name=f"w2s{j}", tag=f"w2s{j}")
                   for j in range(len(S2))]
            w3s = [wpool.tile([P, KH, H], f8e4, name=f"w3s{j}", tag=f"w3s{j}")
                   for j in range(len(S3))]
            KHH = KH // 2
            for half in range(2):
                for j in range(len(S2)):
                    nc.sync.dma_start(
                        w2s[j][:, half * KHH:(half + 1) * KHH, :],
                        din[f"w2s{j}"].ap()[:, half * KHH * H:(half + 1) * KHH * H]
                        .rearrange("p (ko m) -> p ko m", ko=KHH))
            for half in range(2):
                for j in range(len(S3)):
                    nc.sync.dma_start(
                        w3s[j][:, half * KHH:(half + 1) * KHH, :],
                        din[f"w3s{j}"].ap()[:, half * KHH * H:(half + 1) * KHH * H]
                        .rearrange("p (ko m) -> p ko m", ko=KHH))
            w4h8 = wpool.tile([P, KH, AOUT], f8e4, tag="w4h8")
            nc.sync.dma_start(w4h8[:], din["w4h8"].ap().rearrange("p (ko m) -> p ko m", ko=KH))
            w4l8 = wpool.tile([P, KH, AOUT], f8e5, tag="w4l8")
            nc.sync.dma_start(w4l8[:], din["w4l8"].ap().rearrange("p (ko m) -> p ko m", ko=KH))

            # dv1b = x @ W1.T + b1 : f16 hi/lo 3-chain (xh@w1h + xls@w1h +
            # xhs@w1l ~= 2^-22), 4 blocks of 2 c-tiles, one PSUM group each
            for q in range(KH // 2):
                pb = mmps.tile([P, 2, N], f32, name="pp", tag="pp")
                for mov, wst, first, last in ((xh, w1h, True, False),
                                              (xls, w1h, False, False),
                                              (xhs, w1l, False, True)):
                    for k in range(KD):
                        for cc in range(2):
                            c = 2 * q + cc
                            nc.tensor.matmul(pb[:, cc, :], wst[:, k, ts(c, P)],
                                             mov[:, k, :],
                                             start=(first and k == 0),
                                             stop=(last and k == KD - 1))
                for cc in range(2):
                    c = 2 * q + cc
                    nc.scalar.activation(dv1b[:, c, :], pb[:, cc, :], IDENT,
                                         bias=b1sb[:, ts(c, 1)])

        # parity spike-copy set, allocated after the startup pool is freed
        spool2 = ctx.enter_context(tc.tile_pool(name="s1b", bufs=1))
        s1cb = [spool2.tile([P, KH, N], f8e5, name=f"s1cb{j}", tag=f"s1cb{j}")
                for j in range(len(S2))]
        s1sets = [s1c, s1cb]

        nc.vector.memset(v2[:], 0.0)
        nc.gpsimd.memset(v3[:], 0.0)
        ones = vpool.tile([P, 1], f32, tag="ones")

        # step-0 layer-1 spikes (into the even set)
        l1_spike_half(0, s1sets[0], dv1b)
        l1_spike_half(1, s1sets[0], dv1b)

        # ---- helpers ----
        def mm_block(pts, ws, movs, q, kps=None, group=(True, True),
                     defer_last=False):
            nstage = len(ws)
            order = [(kp, j)
                     for kp in (range(KH // 2) if kps is None else kps)
                     for j in range(nstage if not defer_last else nstage - 1)]
            if defer_last:
                order += [(kp, nstage - 1)
                          for kp in (range(KH // 2) if kps is None else kps)]
            last = order[-1]
            for kp, j in order:
                for cc in range(2):
                    c = 2 * q + cc
                    nc.tensor.matmul(pts[:, cc, :],
                                     ws[j][:, 2 * kp:2 * kp + 2, ts(c, P)],
                                     movs[j][:, 2 * kp:2 * kp + 2, :],
                                     start=(group[0] and (kp, j) == order[0]),
                                     stop=(group[1] and (kp, j) == last),
                                     perf_mode=DR)

        def mm_l4_kp(kp):
            for wl in (w4h8, w4l8):
                nc.tensor.matmul(zh[:], wl[:, 2 * kp:2 * kp + 2, :],
                                 s3e[:, 2 * kp:2 * kp + 2, :],
                                 start=False,
                                 stop=(kp == KH // 2 - 1 and wl is w4l8),
                                 skip_group_check=True, perf_mode=DR)

        def mm_l4(t, final):
            for wl in (w4h8, w4l8):
                for kp in range(KH // 2):
                    nc.tensor.matmul(zh[:], wl[:, 2 * kp:2 * kp + 2, :],
                                     s3e[:, 2 * kp:2 * kp + 2, :],
                                     start=(t == T4 and kp == 0 and wl is w4h8),
                                     stop=(t == T - 1 and kp == KH // 2 - 1 and wl is w4l8),
                                     skip_group_check=True, perf_mode=DR)
            if not final:
                nc.vector.tensor_scalar(zh[:], zh[:], 0.5, None, A.mult)

        # ---- the 16-step recurrence ----
        # Emission order per step: L2 blocks emit only updates/spikes/copies
        # (PSUM drains fast), the L1 chain rides the L2->L3 seam, and ALL
        # resets are deferred to the step tail so they soak the next step's
        # DVE slack instead of gating the spike pipeline.
        def l1_half(h, wr):
            hs = slice(4 * h, 4 * h + 4)
            # (v1 * ones) + dv1b: `ones` is rewritten each step from a value the
            # last L2 block produced, so the scheduler cannot hoist this chain
            # ahead of the final spike of layer 2 (greedy engines would
            # otherwise lock out the latency-critical q3 spike).
            nc.vector.scalar_tensor_tensor(v1[:, hs, :], v1[:, hs, :], ones[:],
                                           dv1b[:, hs, :], A.mult, A.add)
            nc.gpsimd.tensor_scalar(wr[0][:, hs, :], v1[:, hs, :], 1.0, None, A.is_ge)
            for j in [jj for jj in CP_ORDER if jj < len(S2)]:
                eng = CP_ENG[j]
                if eng is nc.scalar:
                    eng.activation(wr[j][:, hs, :], wr[0][:, hs, :], IDENT,
                                   scale=float(2.0 ** -S2[j]))
                else:
                    eng.tensor_scalar(wr[j][:, hs, :], wr[0][:, hs, :],
                                      float(2.0 ** -S2[j]), None, A.mult)

        for t in range(T):
            rd = s1sets[t % 2]
            wr = s1sets[(t + 1) % 2]
            if t == T - 1:
                # final step: v2 dead after its spike; threshold 1-v2 goes in
                # the dead v1 tile so the spike reads PSUM directly
                nc.vector.tensor_scalar(v1[:], v2[:], -1.0, 1.0, A.mult, A.add)
            # layer 2: matmuls + updates + spikes + copies (no resets yet)
            for q in range(KH // 2):
                pts2 = mmps.tile([P, 2, N], f32, name="pp", tag="pp")
                mm_block(pts2, w2s, rd, q, defer_last=True)
                qs = slice(2 * q, 2 * q + 2)
                if t == T - 1:
                    for cc in range(2):
                        c = 2 * q + cc
                        nc.vector.scalar_tensor_tensor(s2c[0][:, c, :], pts2[:, cc, :],
                                                       b2sb[:, ts(c, 1)], v1[:, c, :],
                                                       A.add, A.is_ge)
                else:
                    for cc in range(2):
                        c = 2 * q + cc
                        nc.vector.scalar_tensor_tensor(v2[:, c, :], pts2[:, cc, :],
                                                       b2sb[:, ts(c, 1)], v2[:, c, :],
                                                       A.add, A.add)
                    nc.gpsimd.tensor_scalar(s2c[0][:, qs, :], v2[:, qs, :], 1.0, None, A.is_ge)
                for j in range(1, len(S3)):
                    nc.scalar.activation(s2c[j][:, qs, :], s2c[0][:, qs, :], IDENT,
                                         scale=float(2.0 ** -S3[j]))
                if q == KH // 2 - 1 and t < T - 1:
                    nc.vector.tensor_scalar(ones[:], v2[:, KH - 1, 0:1], -3.0e38,
                                            None, A.is_ge)
            if t - 1 >= T4:
                mm_l4(t - 1, final=False)
            # layer-1 spike chain for t+1 (runs under the layer-3 window)
            if t < T - 1:
                l1_half(0, wr)
                l1_half(1, wr)
            else:
                # final step: v3 is dead afterwards, so spike directly from
                # PSUM via (psum+b3) >= (1-v3); threshold lands in the dead
                # dv1b tile during the layer-2 window
                nc.vector.tensor_scalar(dv1b[:], v3[:], -1.0, 1.0, A.mult, A.add)
            # layer 3, staggered: each block's last k-pair is emitted after
            # the next block's first three, so the PE never waits on the
            # freshest layer-2 spikes (same per-group accumulation order)
            NB = KH // 2
            pts3 = [None] * NB

            def l3_open(q):
                pts3[q] = mmps.tile([P, 2, N], f32, name="pp", tag="pp")
                mm_block(pts3[q], w3s, s2c, q, kps=range(NB - 1), group=(True, False))

            def l3_close(q):
                mm_block(pts3[q], w3s, s2c, q, kps=[NB - 1], group=(False, True))
                if t == T - 1:
                    for cc in range(2):
                        c = 2 * q + cc
                        nc.vector.scalar_tensor_tensor(s3e[:, c, :], pts3[q][:, cc, :],
                                                       b3sb[:, ts(c, 1)], dv1b[:, c, :],
                                                       A.add, A.is_ge)
                    return
                for cc in range(2):
                    c = 2 * q + cc
                    nc.vector.scalar_tensor_tensor(v3[:, c, :], pts3[q][:, cc, :],
                                                   b3sb[:, ts(c, 1)], v3[:, c, :],
                                                   A.add, A.add)
                qs = slice(2 * q, 2 * q + 2)
                nc.gpsimd.tensor_scalar(s3e[:, qs, :], v3[:, qs, :], 1.0, None, A.is_ge)

            l3_open(0)
            for q in range(1, NB):
                l3_open(q)
                l3_close(q - 1)
                if t == T - 1 and q > 1:
                    mm_l4_kp(q - 2)
            l3_close(NB - 1)
            if t == T - 1:
                for kp in range(max(0, NB - 2), NB):
                    mm_l4_kp(kp)
            # tail: all resets (drain during the next step's matmul window);
            # at t = T-1 the states are dead, so skip them entirely
            if t < T - 1:
                for q in range(KH // 2):
                    qs = slice(2 * q, 2 * q + 2)
                    nc.vector.scalar_tensor_tensor(v2[:, qs, :], v2[:, qs, :], 1.0,
                                                   s2c[0][:, qs, :], A.min, A.subtract)
                for h in range(2):
                    hs = slice(4 * h, 4 * h + 4)
                    nc.vector.scalar_tensor_tensor(v1[:, hs, :], v1[:, hs, :], 1.0,
                                                   wr[0][:, hs, :], A.min, A.subtract)
                for q in range(KH // 2):
                    qs = slice(2 * q, 2 * q + 2)
                    nc.vector.scalar_tensor_tensor(v3[:, qs, :], v3[:, qs, :], 1.0,
                                                   s3e[:, qs, :], A.min, A.subtract)
        fout = vpool.tile([AOUT, N], f32, tag="fout")
        nc.scalar.activation(fout[:], zh[:], IDENT, scale=0.5, bias=b4sb[:])
        nc.sync.dma_start(dout.ap(), fout[:])

    nc.compile()
    return nc


def _casc_stages(wt, scales):
    """e4m3 residual cascade of wt (f64 accounting, exact host arithmetic)."""
    r = wt.astype(np.float64).copy()
    out = []
    for sb in scales:
        q = (r * 2.0 ** sb).astype(ml_dtypes.float8_e4m3)
        out.append(q)
        r -= q.astype(np.float64) * 2.0 ** -sb
    return out


def _km(a, ko):
    # (ko*P, m) -> (P, ko*m): partition-major layout matching the SBUF tiles
    m = a.shape[1]
    return np.ascontiguousarray(a.reshape(ko, P, m).transpose(1, 0, 2).reshape(P, ko * m))


def _hilo(a):
    hi = a.astype(np.float16)
    lo = ((a.astype(np.float32) - hi.astype(np.float32)) * np.float32(2.0 ** 11)
          ).astype(np.float16)
    return hi, lo


def _prep_inputs(x, W1, b1, W2, b2, W3, b3, W4, b4):
    xT = np.ascontiguousarray(x.T.astype(np.float32))          # (D, B)
    xh_f, xl_f = _hilo(xT)
    xls_f = (xl_f.astype(np.float32) * np.float32(2.0 ** -11)).astype(np.float16)
    xhs_f = (xh_f.astype(np.float32) * np.float32(2.0 ** -11)).astype(np.float16)
    w1h_f, w1l_f = _hilo(np.ascontiguousarray(W1.T).astype(np.float32))
    w2t = np.ascontiguousarray(W2.T).astype(np.float32)        # (H, H)
    w3t = np.ascontiguousarray(W3.T).astype(np.float32)
    w4t = np.ascontiguousarray(W4.T).astype(np.float32)        # (H, AOUT)
    w4h8 = w4t.astype(ml_dtypes.float8_e4m3)
    w4l8 = (w4t - w4h8.astype(np.float32)).astype(ml_dtypes.float8_e5m2)

    shared = {"w1h": _km(w1h_f, KD), "w1l": _km(w1l_f, KD)}
    for j, q in enumerate(_casc_stages(w2t, S2)):
        shared[f"w2s{j}"] = _km(q, KH)
    for j, q in enumerate(_casc_stages(w3t, S3)):
        shared[f"w3s{j}"] = _km(q, KH)
    shared.update({
        "w4h8": _km(w4h8, KH), "w4l8": _km(w4l8, KH),
        "b1": np.ascontiguousarray(b1.reshape(KH, P).T.astype(np.float32)),
        "b2": np.ascontiguousarray(b2.reshape(KH, P).T.astype(np.float32)),
        "b3": np.ascontiguousarray(b3.reshape(KH, P).T.astype(np.float32)),
        "b4f": ((1.0 - 2.0 ** -T) * b4).astype(np.float32).reshape(AOUT, 1),
    })
    in_maps = []
    for i in range(NCORES):
        m = dict(shared)
        m["xh"] = _km(xh_f[:, i * N:(i + 1) * N], KD)
        m["xls"] = _km(xls_f[:, i * N:(i + 1) * N], KD)
        m["xhs"] = _km(xhs_f[:, i * N:(i + 1) * N], KD)
        in_maps.append(m)
    return in_maps


def _run(in_maps):
    from concourse.bass_utils import run_bass_kernel_spmd
    if "nc" not in _CACHE:
        _CACHE["nc"] = _build()
    res = run_bass_kernel_spmd(_CACHE["nc"], in_maps, list(range(NCORES)))
    parts = [res.results[i]["v4T"] for i in range(NCORES)]     # each (AOUT, N)
    return np.ascontiguousarray(np.concatenate(parts, axis=1).T).astype(np.float32)


def kernel(x, W1, b1, W2, b2, W3, b3, W4, b4):
    in_maps = _prep_inputs(x, W1, b1, W2, b2, W3, b3, W4, b4)
    return _run(in_maps)
